# revision 8
# baseline (speedup 1.0000x reference)
"""Trainium2 Bass kernel for a 2-layer GCN encoder + dot-product link decoder.

Model (PyG-style GCNConv with self-loops and symmetric normalization):
    h  = relu(gcn(x, W1, b1));  z = gcn(h, W2, b2)
    logits[k] = sum(z[ei0[k]] * z[ei1[k]])  over pos+neg edge pairs

Distribution over 8 NeuronCores: nodes block-sharded; per core
  - h1' = (x_shard @ W1) * dinv  (bf16 GEMM), AllGather -> full bf16 table,
  - aggregation: per (dst-block, src-segment) group, dma_gather of in-edge
    source rows (int16 idx, 4 row segments, 4 SWDGE queues round-robin),
    one-hot segment-sum matmuls accumulated in PSUM (all bf16 operands),
    self-loop block row added; epilogues on the Scalar engine,
  - layer-2 GEMM fused into the layer-1 epilogue via PE transpose; the
    64-wide tables (h2', z) are stored column-duplicated [v|v] so gather
    rows stay 256B (hardware requires elem bytes % 256 == 0),
  - decode: pairs grouped into 16 (seg(e0), seg(e1)) classes, gathers in
    <=4096-idx chunks, DVE multiply (bf16 2x) + reduce over first half.

Everything bf16 except PSUM accumulation and the final logit reduction
(f32). Validated ~8e-4 rel err vs f32 reference.
"""
import sys

sys.path.insert(0, "/opt/trn_rl_repo")

import numpy as np
import ml_dtypes

import concourse.bass as bass
import concourse.bacc as bacc
import concourse.mybir as mybir
import concourse.tile as tile
from concourse.bass_utils import run_bass_kernel_spmd

BF16 = ml_dtypes.bfloat16
P = 128
NCORES = 8
NSEG = 4          # int16 gather indices -> table split into 4 row segments
SUPB = 2          # dst blocks per gather super-group
DCHUNK = 4096     # decode gather indices per call


def wrap_idx(flat, n):
    """Pack flat int indices (len n, multiple of 128) into the dma_gather
    SBUF layout: [16, n/16] with index i at [i % 16, i // 16], replicated
    to 128 partitions."""
    a = np.asarray(flat, np.int16).reshape(n // 16, 16).T  # [16, n/16]
    return np.tile(a, (8, 1)).copy()  # [128, n/16]


# ---------------------------------------------------------------- host side


def preprocess(x, train_pos_edge_index, pos_edge_index, neg_edge_index, W1, b1, W2, b2):
    N, F1 = x.shape
    H1 = W1.shape[1]
    F2 = W2.shape[1]
    assert N % NCORES == 0, N
    nsh = N // NCORES
    csh = ((nsh + P - 1) // P) * P
    nblk = csh // P
    assert nblk % SUPB == 0, nblk
    nsup = nblk // SUPB
    ntot = NCORES * csh
    assert ntot % NSEG == 0
    segsz = ntot // NSEG
    assert segsz <= 32767, segsz

    src_o = np.asarray(train_pos_edge_index[0], dtype=np.int64)
    dst_o = np.asarray(train_pos_edge_index[1], dtype=np.int64)

    def remap(a):
        return (a // nsh) * csh + (a % nsh)

    src_n = remap(src_o)
    dst_n = remap(dst_o)

    deg = np.bincount(dst_o, minlength=N).astype(np.float64) + 1.0
    dinv_o = (1.0 / np.sqrt(deg)).astype(np.float32)
    dinv = np.zeros(ntot, np.float32)
    dinv[remap(np.arange(N, dtype=np.int64))] = dinv_o

    # ---- edges grouped by (dst block, src segment)
    blk = dst_n // P                      # global dst block
    seg = src_n // segsz
    nblk_tot = ntot // P
    key = blk * NSEG + seg
    order = np.argsort(key, kind="stable")
    key_s = key[order]
    srcloc_s = (src_n[order] % segsz).astype(np.int16)
    dstloc_s = (dst_n[order] % P).astype(np.float32)
    ngrp = nblk_tot * NSEG
    counts = np.bincount(key_s, minlength=ngrp)
    tbs = int(np.ceil(counts.max() / P))  # tiles per (block, seg), uniform
    gsl = tbs * P                         # slots per (block, seg)
    starts = np.concatenate([[0], np.cumsum(counts)])
    within = np.arange(len(key_s)) - starts[key_s]
    flat = key_s * gsl + within

    sidx_arr = np.zeros(ngrp * gsl, np.int16)          # pad -> row 0 of segment
    dloc_arr = np.full(ngrp * gsl, -1.0, np.float32)   # pad -> no dst
    sidx_arr[flat] = srcloc_s
    dloc_arr[flat] = dstloc_s
    # reorganize to supers: [nsup_tot][seg][b][tile][p]
    sidx_arr = sidx_arr.reshape(nblk_tot, NSEG, gsl)
    dloc_arr = dloc_arr.reshape(nblk_tot, NSEG, gsl)
    nsup_tot = nblk_tot // SUPB
    sidx_sup = sidx_arr.reshape(nsup_tot, SUPB, NSEG, gsl).transpose(0, 2, 1, 3)
    sidx_sup = sidx_sup.reshape(nsup_tot, NSEG, SUPB * gsl)
    dloc_sup = dloc_arr.reshape(nsup_tot, SUPB, NSEG, gsl).transpose(0, 2, 1, 3)
    dloc_sup = dloc_sup.reshape(nsup_tot, NSEG, SUPB * gsl)

    n_call = SUPB * gsl                   # indices per gather call
    sidx_dev = np.zeros((nsup_tot, NSEG, P, n_call // 16), np.int16)
    for u in range(nsup_tot):
        for s in range(NSEG):
            sidx_dev[u, s] = wrap_idx(sidx_sup[u, s], n_call)
    sidx_dev = sidx_dev.transpose(0, 2, 1, 3).copy()  # [nsup, P, NSEG, C]
    nt_sup = NSEG * SUPB * tbs            # gather tiles per super
    dloc_dev = (
        dloc_sup.reshape(nsup_tot, NSEG * SUPB * tbs, P).transpose(0, 2, 1)
    ).astype(BF16).copy()  # [nsup_tot, 128, nt_sup] bf16 (-1..127 exact)

    # ---- decode pairs grouped into 16 (seg0, seg1) classes per core
    ei = np.concatenate(
        [np.asarray(pos_edge_index), np.asarray(neg_edge_index)], axis=1
    ).astype(np.int64)
    ep = ei.shape[1]
    ndec = (ep + NCORES - 1) // NCORES
    e0 = remap(ei[0])
    e1 = remap(ei[1])
    ncls = NSEG * NSEG
    cls_of = (e0 // segsz) * NSEG + (e1 // segsz)
    tcls = 0
    core_cls = []
    for c in range(NCORES):
        lo, hi = c * ndec, min((c + 1) * ndec, ep)
        k = cls_of[lo:hi]
        cnt = np.bincount(k, minlength=ncls)
        tcls = max(tcls, int(np.ceil(cnt.max() / P)))
        core_cls.append((lo, hi, k))
    dsl = tcls * P                        # slots per class
    # decode gathers split into DCHUNK-idx calls, each wrapped independently
    ndch = (dsl + DCHUNK - 1) // DCHUNK
    dslp = ndch * DCHUNK                  # padded to whole chunks
    d0idx = np.zeros((NCORES, ncls, ndch, P, DCHUNK // 16), np.int16)
    d1idx = np.zeros((NCORES, ncls, ndch, P, DCHUNK // 16), np.int16)
    slot_pair = np.full((NCORES, ncls * dsl), -1, np.int64)  # -> original pair
    for c in range(NCORES):
        lo, hi, k = core_cls[c]
        o = np.argsort(k, kind="stable")
        cnt = np.bincount(k, minlength=ncls)
        st = np.concatenate([[0], np.cumsum(cnt)])
        for kk in range(ncls):
            sel = o[st[kk] : st[kk + 1]] + lo
            i0 = np.zeros(dslp, np.int16)
            i1 = np.zeros(dslp, np.int16)
            i0[: len(sel)] = (e0[sel] % segsz).astype(np.int16)
            i1[: len(sel)] = (e1[sel] % segsz).astype(np.int16)
            for ch in range(ndch):
                d0idx[c, kk, ch] = wrap_idx(i0[ch * DCHUNK : (ch + 1) * DCHUNK], DCHUNK)
                d1idx[c, kk, ch] = wrap_idx(i1[ch * DCHUNK : (ch + 1) * DCHUNK], DCHUNK)
            slot_pair[c, kk * dsl : kk * dsl + len(sel)] = sel

    iota_bf = np.broadcast_to(np.arange(P, dtype=np.float32).astype(BF16), (P, P)).copy()
    ident_bf = np.eye(P, dtype=np.float32).astype(BF16)
    b1rep = np.broadcast_to(np.asarray(b1, np.float32), (P, H1)).copy()
    b2rep = np.broadcast_to(
        np.concatenate([np.asarray(b2, np.float32)] * 2), (P, 2 * F2)
    ).copy()
    W1b = np.asarray(W1, np.float32).astype(BF16)
    W2dup = np.concatenate([np.asarray(W2, np.float32)] * 2, axis=1).astype(BF16)

    x = np.asarray(x, np.float32)
    in_maps = []
    for c in range(NCORES):
        xs = np.zeros((csh, F1), np.float32)
        xs[:nsh] = x[c * nsh : (c + 1) * nsh]
        dinv_sh = dinv[c * csh : (c + 1) * csh]
        in_maps.append(
            {
                "xT": xs.T.astype(BF16).copy(),
                "dinvT": dinv_sh.reshape(nblk, P).T.copy(),
                "W1": W1b,
                "W2dup": W2dup,
                "b1rep": b1rep,
                "b2rep": b2rep,
                "iota_bf": iota_bf,
                "ident_bf": ident_bf,
                "sidx": sidx_dev[c * nsup : (c + 1) * nsup],
                "dloc": dloc_dev[c * nsup : (c + 1) * nsup],
                "d0idx": d0idx[c].transpose(0, 2, 1, 3).copy(),
                "d1idx": d1idx[c].transpose(0, 2, 1, 3).copy(),
            }
        )
    meta = dict(
        N=N, F1=F1, H1=H1, F2=F2, nsh=nsh, csh=csh, nblk=nblk, ntot=ntot,
        segsz=segsz, nsup=nsup, tbs=tbs, gsl=gsl, n_call=n_call, nt_sup=nt_sup,
        ncls=ncls, tcls=tcls, dsl=dsl, ndch=ndch, ndec=ndec, ep=ep,
        has_b1=bool(np.any(np.asarray(b1))), has_b2=bool(np.any(np.asarray(b2))),
    )
    return in_maps, meta, slot_pair


# -------------------------------------------------------------- device side


def build(meta, debug=False):
    f32 = mybir.dt.float32
    bf16 = mybir.dt.bfloat16
    i16 = mybir.dt.int16
    csh, nblk, ntot, segsz = meta["csh"], meta["nblk"], meta["ntot"], meta["segsz"]
    F1, H1, F2 = meta["F1"], meta["H1"], meta["F2"]
    nsup, tbs, n_call, nt_sup = meta["nsup"], meta["tbs"], meta["n_call"], meta["nt_sup"]
    ncls, tcls, dsl, ndch = meta["ncls"], meta["tcls"], meta["dsl"], meta["ndch"]
    has_b1, has_b2 = meta["has_b1"], meta["has_b2"]
    F2d = 2 * F2
    AF = mybir.ActivationFunctionType
    nper = SUPB * tbs                     # tiles per (super, seg)

    nc = bacc.Bacc(
        "TRN2", target_bir_lowering=False, debug=debug, num_devices=NCORES,
        num_swdge_queues=4,
    )
    qn = [0]

    def next_q():
        qn[0] = (qn[0] + 1) % 4
        return qn[0]

    xT = nc.dram_tensor("xT", [F1, csh], bf16, kind="ExternalInput")
    dinvT = nc.dram_tensor("dinvT", [P, nblk], f32, kind="ExternalInput")
    W1 = nc.dram_tensor("W1", [F1, H1], bf16, kind="ExternalInput")
    W2dup = nc.dram_tensor("W2dup", [H1, F2d], bf16, kind="ExternalInput")
    b1rep = nc.dram_tensor("b1rep", [P, H1], f32, kind="ExternalInput")
    b2rep = nc.dram_tensor("b2rep", [P, F2d], f32, kind="ExternalInput")
    iota_bf = nc.dram_tensor("iota_bf", [P, P], bf16, kind="ExternalInput")
    ident_bf = nc.dram_tensor("ident_bf", [P, P], bf16, kind="ExternalInput")
    sidx = nc.dram_tensor("sidx", [nsup, P, NSEG, n_call // 16], i16, kind="ExternalInput")
    dloc = nc.dram_tensor("dloc", [nsup, P, nt_sup], bf16, kind="ExternalInput")
    d0idx = nc.dram_tensor("d0idx", [ncls, P, ndch, DCHUNK // 16], i16, kind="ExternalInput")
    d1idx = nc.dram_tensor("d1idx", [ncls, P, ndch, DCHUNK // 16], i16, kind="ExternalInput")
    logits = nc.dram_tensor("logits", [ncls, P, tcls], f32, kind="ExternalOutput")

    h1_shard = nc.dram_tensor("h1_shard", [csh, H1], bf16)
    h1_full = nc.dram_tensor("h1_full", [ntot, H1], bf16, addr_space="Shared")
    h2_shard = nc.dram_tensor("h2_shard", [csh, F2d], bf16)
    h2_full = nc.dram_tensor("h2_full", [ntot, F2d], bf16, addr_space="Shared")
    z_shard = nc.dram_tensor("z_shard", [csh, F2d], bf16)
    z_full = nc.dram_tensor("z_full", [ntot, F2d], bf16, addr_space="Shared")

    rg = [list(range(NCORES))]

    def aggregate(tc, full_tbl, shard_tbl, F, dinv_t, out_tbl, Fout, emit):
        """Per super: one idx DMA + NSEG dma_gathers (per-seg tiles, rr
        queues) + one-hot segment matmuls into per-block PSUM, self-loop
        add, emit(b, t0, epool, ot, bb) filling a per-super staging tile,
        flushed with one DMA to out_tbl."""
        ic = n_call // 16
        with (
            tc.tile_pool(name="idx", bufs=6) as ipool,
            tc.tile_pool(name="gath", bufs=6) as gpool,
            tc.tile_pool(name="oneh", bufs=3) as spool,
            tc.tile_pool(name="dlp", bufs=3) as dpool,
            tc.tile_pool(name="selfl", bufs=4) as hpool,
            tc.tile_pool(name="epi", bufs=4) as epool,
            tc.tile_pool(name="outst", bufs=3) as opool,
            tc.tile_pool(name="ps_acc", bufs=2, space="PSUM") as pacc,
        ):
            for u in range(nsup):
                it = ipool.tile([P, NSEG * ic], i16, tag="it")
                nc.sync.dma_start(
                    out=it[:], in_=sidx[u].rearrange("p s c -> p (s c)")
                )
                gs = []
                for s in range(NSEG):
                    G = gpool.tile([P, nper * F], bf16, tag=f"G{s}")
                    nc.gpsimd.dma_gather(
                        G[:].rearrange("p (t f) -> p t f", t=nper),
                        full_tbl[s * segsz : (s + 1) * segsz, :],
                        it[:, s * ic : (s + 1) * ic],
                        n_call,
                        n_call,
                        F,
                        single_packet=False,
                        queue_num=next_q(),
                    )
                    gs.append(G)
                dt = dpool.tile([P, nt_sup], bf16, tag="dt")
                nc.sync.dma_start(out=dt[:], in_=dloc[u, :, :])
                S = spool.tile([P, nt_sup * P], bf16, tag="S")
                nc.vector.tensor_tensor(
                    out=S[:].rearrange("p (t j) -> p t j", t=nt_sup),
                    in0=dt[:, :, None].to_broadcast([P, nt_sup, P]),
                    in1=iota_t[:, None, :].to_broadcast([P, nt_sup, P]),
                    op=mybir.AluOpType.is_equal,
                )
                hb = hpool.tile([P, SUPB * F], bf16, tag="hblk")
                nc.sync.dma_start(
                    out=hb[:].rearrange("p (b f) -> p b f", b=SUPB),
                    in_=shard_tbl[u * SUPB * P : (u + 1) * SUPB * P, :]
                    .rearrange("(b p) f -> p b f", b=SUPB),
                )
                ot = opool.tile([P, SUPB * Fout], bf16, tag="ot")
                for bb in range(SUPB):
                    b = u * SUPB + bb
                    acc = pacc.tile([P, F], f32, tag=f"acc{bb}")
                    n_i = 0
                    for s in range(NSEG):
                        for j in range(tbs):
                            c = bb * tbs + j
                            t = s * nper + c
                            nc.tensor.matmul(
                                out=acc[:],
                                lhsT=S[:, t * P : (t + 1) * P],
                                rhs=gs[s][:, c * F : (c + 1) * F],
                                start=(n_i == 0),
                                stop=False,
                            )
                            n_i += 1
                    # self-loop row add fused into the PSUM group: I^T @ hb
                    nc.tensor.matmul(
                        out=acc[:],
                        lhsT=ident[:],
                        rhs=hb[:, bb * F : (bb + 1) * F],
                        start=False,
                        stop=True,
                    )
                    emit(u * SUPB + bb, acc, epool, ot, bb)
                nc.sync.dma_start(
                    out=out_tbl[u * SUPB * P : (u + 1) * SUPB * P, :]
                    .rearrange("(b p) f -> p b f", b=SUPB),
                    in_=ot[:].rearrange("p (b f) -> p b f", b=SUPB),
                )

    with tile.TileContext(nc) as tc:
        with tc.tile_pool(name="const", bufs=1) as cpool:
            W1_t = cpool.tile([F1, H1], bf16, tag="w1")
            nc.sync.dma_start(out=W1_t[:], in_=W1[:])
            W2_t = cpool.tile([H1, F2d], bf16, tag="w2")
            nc.sync.dma_start(out=W2_t[:], in_=W2dup[:])
            b1_t = cpool.tile([P, H1], f32, tag="b1")
            nc.sync.dma_start(out=b1_t[:], in_=b1rep[:])
            b2_t = cpool.tile([P, F2d], f32, tag="b2")
            nc.sync.dma_start(out=b2_t[:], in_=b2rep[:])
            dinv_t = cpool.tile([P, nblk], f32, tag="dinv")
            nc.sync.dma_start(out=dinv_t[:], in_=dinvT[:])
            ident = cpool.tile([P, P], bf16, tag="ident")
            nc.sync.dma_start(out=ident[:], in_=ident_bf[:])
            iota_t = cpool.tile([P, P], bf16, tag="iotaf")
            nc.sync.dma_start(out=iota_t[:], in_=iota_bf[:])

            # ---------------- phase A: h1' = (x @ W1) * dinv (sharded GEMM)
            with (
                tc.tile_pool(name="gemm1", bufs=3) as gp,
                tc.tile_pool(name="gemm1x", bufs=1) as gx,
                tc.tile_pool(name="ps_a", bufs=4, space="PSUM") as pa,
            ):
                xT_t = gx.tile([F1, csh], bf16, tag="xT")
                nc.sync.dma_start(out=xT_t[:], in_=xT[:])
                for i in range(nblk):
                    ps = pa.tile([P, H1], f32, tag="psA")
                    nc.tensor.matmul(
                        out=ps[:],
                        lhsT=xT_t[:, i * P : (i + 1) * P],
                        rhs=W1_t[:],
                        start=True,
                        stop=True,
                    )
                    ht = gp.tile([P, H1], bf16, tag="h1t")
                    nc.scalar.activation(
                        out=ht[:], in_=ps[:], func=AF.Copy,
                        scale=dinv_t[:, i : i + 1],
                    )
                    nc.sync.dma_start(
                        out=h1_shard[i * P : (i + 1) * P, :], in_=ht[:]
                    )

            nc.gpsimd.collective_compute(
                "AllGather",
                mybir.AluOpType.bypass,
                ins=[h1_shard.ap().opt()],
                outs=[h1_full.ap().opt()],
                replica_groups=rg,
            )

            # ---------------- phase C: layer-1 aggregation + fused GEMM2
            with (
                tc.tile_pool(name="ps_tr", bufs=2, space="PSUM") as ptr,
                tc.tile_pool(name="ps_h2", bufs=2, space="PSUM") as ph2,
            ):

                def emit1(b, t0, epool, ot, bb):
                    o1 = epool.tile([P, H1], bf16, tag="o1")
                    if has_b1:
                        t1 = epool.tile([P, H1], f32, tag="t1")
                        nc.vector.tensor_scalar_mul(
                            t1[:], t0[:], dinv_t[:, b : b + 1]
                        )
                        t2 = epool.tile([P, H1], f32, tag="t2")
                        nc.vector.tensor_tensor(
                            out=t2[:], in0=t1[:], in1=b1_t[:], op=mybir.AluOpType.add
                        )
                        nc.scalar.activation(out=o1[:], in_=t2[:], func=AF.Relu)
                    else:
                        nc.scalar.activation(
                            out=o1[:], in_=t0[:], func=AF.Relu,
                            scale=dinv_t[:, b : b + 1],
                        )
                    tp = ptr.tile([H1, P], bf16, tag="tp")
                    nc.tensor.transpose(out=tp[:], in_=o1[:], identity=ident[:])
                    o1T = epool.tile([H1, P], bf16, tag="o1T")
                    nc.scalar.activation(out=o1T[:], in_=tp[:], func=AF.Copy)
                    hp = ph2.tile([P, F2d], f32, tag="hp")
                    nc.tensor.matmul(
                        out=hp[:], lhsT=o1T[:], rhs=W2_t[:], start=True, stop=True
                    )
                    nc.scalar.activation(
                        out=ot[:, bb * F2d : (bb + 1) * F2d], in_=hp[:],
                        func=AF.Copy, scale=dinv_t[:, b : b + 1],
                    )

                aggregate(tc, h1_full, h1_shard, H1, dinv_t, h2_shard, F2d, emit1)

            nc.gpsimd.collective_compute(
                "AllGather",
                mybir.AluOpType.bypass,
                ins=[h2_shard.ap().opt()],
                outs=[h2_full.ap().opt()],
                replica_groups=rg,
            )

            # ---------------- phase E: layer-2 aggregation -> z (dup cols)
            def emit2(b, t0, epool, ot, bb):
                if has_b2:
                    t1 = epool.tile([P, F2d], f32, tag="t1z")
                    nc.vector.tensor_scalar_mul(t1[:], t0[:], dinv_t[:, b : b + 1])
                    t2 = epool.tile([P, F2d], f32, tag="t2z")
                    nc.vector.tensor_tensor(
                        out=t2[:], in0=t1[:], in1=b2_t[:], op=mybir.AluOpType.add
                    )
                    nc.vector.tensor_copy(
                        out=ot[:, bb * F2d : (bb + 1) * F2d], in_=t2[:]
                    )
                else:
                    nc.scalar.activation(
                        out=ot[:, bb * F2d : (bb + 1) * F2d], in_=t0[:],
                        func=AF.Copy, scale=dinv_t[:, b : b + 1],
                    )

            aggregate(tc, h2_full, h2_shard, F2d, dinv_t, z_shard, F2d, emit2)

            nc.gpsimd.collective_compute(
                "AllGather",
                mybir.AluOpType.bypass,
                ins=[z_shard.ap().opt()],
                outs=[z_full.ap().opt()],
                replica_groups=rg,
            )

            # ---------------- phase G: decode (16 classes, chunked gathers)
            with (
                tc.tile_pool(name="didx", bufs=8) as ipool,
                tc.tile_pool(name="dz", bufs=2) as zpool,
                tc.tile_pool(name="dm", bufs=2) as mpool,
                tc.tile_pool(name="dl", bufs=3) as lpool,
            ):
                cpt = DCHUNK // P         # tiles per chunk
                dc = DCHUNK // 16
                for k in range(ncls):
                    s0, s1 = k // NSEG, k % NSEG
                    Z0 = zpool.tile([P, tcls * F2d], bf16, tag="Z0")
                    Z1 = zpool.tile([P, tcls * F2d], bf16, tag="Z1")
                    i0 = ipool.tile([P, ndch * dc], i16, tag="i0")
                    nc.sync.dma_start(
                        out=i0[:], in_=d0idx[k].rearrange("p h c -> p (h c)")
                    )
                    i1 = ipool.tile([P, ndch * dc], i16, tag="i1")
                    nc.sync.dma_start(
                        out=i1[:], in_=d1idx[k].rearrange("p h c -> p (h c)")
                    )
                    for ch in range(ndch):
                        nt_ch = min(cpt, tcls - ch * cpt)
                        n_i = nt_ch * P
                        nc.gpsimd.dma_gather(
                            Z0[:, ch * cpt * F2d : (ch * cpt + nt_ch) * F2d]
                            .rearrange("p (t f) -> p t f", t=nt_ch),
                            z_full[s0 * segsz : (s0 + 1) * segsz, :],
                            i0[:, ch * dc : (ch + 1) * dc], n_i, n_i, F2d,
                            single_packet=False, queue_num=next_q(),
                        )
                        nc.gpsimd.dma_gather(
                            Z1[:, ch * cpt * F2d : (ch * cpt + nt_ch) * F2d]
                            .rearrange("p (t f) -> p t f", t=nt_ch),
                            z_full[s1 * segsz : (s1 + 1) * segsz, :],
                            i1[:, ch * dc : (ch + 1) * dc], n_i, n_i, F2d,
                            single_packet=False, queue_num=next_q(),
                        )
                    M = mpool.tile([P, tcls * F2d], bf16, tag="M")
                    nc.vector.tensor_tensor(
                        out=M[:], in0=Z0[:], in1=Z1[:], op=mybir.AluOpType.mult
                    )
                    L = lpool.tile([P, tcls], f32, tag="L")
                    nc.vector.tensor_reduce(
                        out=L[:],
                        in_=M[:].rearrange("p (t f) -> p t f", t=tcls)[:, :, 0:F2],
                        axis=mybir.AxisListType.X,
                        op=mybir.AluOpType.add,
                    )
                    nc.sync.dma_start(out=logits[k, :, :], in_=L[:])

    nc.compile()
    return nc


# -------------------------------------------------------------------- entry


def assemble_logits(results, meta, slot_pair):
    ep = meta["ep"]
    ncls, dsl, tcls = meta["ncls"], meta["dsl"], meta["tcls"]
    logits = np.empty(ep, np.float32)
    for c in range(len(results)):
        lg = results[c]["logits"]  # [ncls, P, tcls]
        vals = lg.transpose(0, 2, 1).reshape(ncls * dsl)  # pos i = j*128+p
        sp = slot_pair[c]
        m = sp >= 0
        logits[sp[m]] = vals[m]
    return logits


def kernel(**inputs) -> np.ndarray:
    in_maps, meta, slot_pair = preprocess(**inputs)
    nc = build(meta)
    res = run_bass_kernel_spmd(nc, in_maps, core_ids=list(range(NCORES)))
    return assemble_logits(res.results, meta, slot_pair)


# revision 9
# speedup vs baseline: 1.0602x; 1.0602x over previous
"""Trainium2 Bass kernel for a 2-layer GCN encoder + dot-product link decoder.

Model (PyG-style GCNConv with self-loops and symmetric normalization):
    h  = relu(gcn(x, W1, b1));  z = gcn(h, W2, b2)
    logits[k] = sum(z[ei0[k]] * z[ei1[k]])  over pos+neg edge pairs

Distribution over 8 NeuronCores: nodes block-sharded; per core
  - h1' = (x_shard @ W1) * dinv  (bf16 GEMM), AllGather -> full bf16 table,
  - aggregation: per (dst-block, src-segment) group, dma_gather of in-edge
    source rows (int16 idx, 4 row segments, 4 SWDGE queues round-robin),
    one-hot segment-sum matmuls accumulated in PSUM (all bf16 operands),
    self-loop block row added; epilogues on the Scalar engine,
  - layer-2 GEMM fused into the layer-1 epilogue via PE transpose; the
    64-wide tables (h2', z) are stored column-duplicated [v|v] so gather
    rows stay 256B (hardware requires elem bytes % 256 == 0),
  - decode: pairs grouped into 16 (seg(e0), seg(e1)) classes, gathers in
    <=4096-idx chunks, DVE multiply (bf16 2x) + reduce over first half.

Everything bf16 except PSUM accumulation and the final logit reduction
(f32). Validated ~8e-4 rel err vs f32 reference.
"""
import sys

sys.path.insert(0, "/opt/trn_rl_repo")

import numpy as np
import ml_dtypes

import concourse.bass as bass
import concourse.bacc as bacc
import concourse.mybir as mybir
import concourse.tile as tile
from concourse.bass_utils import run_bass_kernel_spmd

BF16 = ml_dtypes.bfloat16
P = 128
NCORES = 8
NSEG = 4          # int16 gather indices -> table split into 4 row segments
SUPB = 2          # dst blocks per gather super-group
DCHUNK = 4096     # decode gather indices per call


def wrap_idx(flat, n):
    """Pack flat int indices (len n, multiple of 128) into the dma_gather
    SBUF layout: [16, n/16] with index i at [i % 16, i // 16], replicated
    to 128 partitions."""
    a = np.asarray(flat, np.int16).reshape(n // 16, 16).T  # [16, n/16]
    return np.tile(a, (8, 1)).copy()  # [128, n/16]


# ---------------------------------------------------------------- host side


def preprocess(x, train_pos_edge_index, pos_edge_index, neg_edge_index, W1, b1, W2, b2):
    N, F1 = x.shape
    H1 = W1.shape[1]
    F2 = W2.shape[1]
    assert N % NCORES == 0, N
    nsh = N // NCORES
    csh = ((nsh + P - 1) // P) * P
    nblk = csh // P
    assert nblk % SUPB == 0, nblk
    nsup = nblk // SUPB
    ntot = NCORES * csh
    assert ntot % NSEG == 0
    segsz = ntot // NSEG
    assert segsz <= 32767, segsz

    src_o = np.asarray(train_pos_edge_index[0], dtype=np.int64)
    dst_o = np.asarray(train_pos_edge_index[1], dtype=np.int64)

    def remap(a):
        return (a // nsh) * csh + (a % nsh)

    src_n = remap(src_o)
    dst_n = remap(dst_o)

    deg = np.bincount(dst_o, minlength=N).astype(np.float64) + 1.0
    dinv_o = (1.0 / np.sqrt(deg)).astype(np.float32)
    dinv = np.zeros(ntot, np.float32)
    dinv[remap(np.arange(N, dtype=np.int64))] = dinv_o

    # ---- edges grouped by (dst block, src segment)
    blk = dst_n // P                      # global dst block
    seg = src_n // segsz
    nblk_tot = ntot // P
    key = blk * NSEG + seg
    order = np.argsort(key, kind="stable")
    key_s = key[order]
    srcloc_s = (src_n[order] % segsz).astype(np.int16)
    dstloc_s = (dst_n[order] % P).astype(np.float32)
    ngrp = nblk_tot * NSEG
    counts = np.bincount(key_s, minlength=ngrp)
    tbs = int(np.ceil(counts.max() / P))  # tiles per (block, seg), uniform
    gsl = tbs * P                         # slots per (block, seg)
    starts = np.concatenate([[0], np.cumsum(counts)])
    within = np.arange(len(key_s)) - starts[key_s]
    flat = key_s * gsl + within

    sidx_arr = np.zeros(ngrp * gsl, np.int16)          # pad -> row 0 of segment
    dloc_arr = np.full(ngrp * gsl, -1.0, np.float32)   # pad -> no dst
    sidx_arr[flat] = srcloc_s
    dloc_arr[flat] = dstloc_s
    # reorganize to supers: [nsup_tot][seg][b][tile][p]
    sidx_arr = sidx_arr.reshape(nblk_tot, NSEG, gsl)
    dloc_arr = dloc_arr.reshape(nblk_tot, NSEG, gsl)
    nsup_tot = nblk_tot // SUPB
    sidx_sup = sidx_arr.reshape(nsup_tot, SUPB, NSEG, gsl).transpose(0, 2, 1, 3)
    sidx_sup = sidx_sup.reshape(nsup_tot, NSEG, SUPB * gsl)
    dloc_sup = dloc_arr.reshape(nsup_tot, SUPB, NSEG, gsl).transpose(0, 2, 1, 3)
    dloc_sup = dloc_sup.reshape(nsup_tot, NSEG, SUPB * gsl)

    n_call = SUPB * gsl                   # indices per gather call
    sidx_dev = np.zeros((nsup_tot, NSEG, P, n_call // 16), np.int16)
    for u in range(nsup_tot):
        for s in range(NSEG):
            sidx_dev[u, s] = wrap_idx(sidx_sup[u, s], n_call)
    sidx_dev = sidx_dev.transpose(0, 2, 1, 3).copy()  # [nsup, P, NSEG, C]
    nt_sup = NSEG * SUPB * tbs            # gather tiles per super
    dloc_dev = (
        dloc_sup.reshape(nsup_tot, NSEG * SUPB * tbs, P).transpose(0, 2, 1)
    ).astype(BF16).copy()  # [nsup_tot, 128, nt_sup] bf16 (-1..127 exact)

    # ---- decode pairs grouped into 16 (seg0, seg1) classes per core
    ei = np.concatenate(
        [np.asarray(pos_edge_index), np.asarray(neg_edge_index)], axis=1
    ).astype(np.int64)
    ep = ei.shape[1]
    ndec = (ep + NCORES - 1) // NCORES
    e0 = remap(ei[0])
    e1 = remap(ei[1])
    ncls = NSEG * NSEG
    cls_of = (e0 // segsz) * NSEG + (e1 // segsz)
    tcls = 0
    core_cls = []
    for c in range(NCORES):
        lo, hi = c * ndec, min((c + 1) * ndec, ep)
        k = cls_of[lo:hi]
        cnt = np.bincount(k, minlength=ncls)
        tcls = max(tcls, int(np.ceil(cnt.max() / P)))
        core_cls.append((lo, hi, k))
    dsl = tcls * P                        # slots per class
    # decode gathers split into DCHUNK-idx calls, each wrapped independently
    ndch = (dsl + DCHUNK - 1) // DCHUNK
    dslp = ndch * DCHUNK                  # padded to whole chunks
    d0idx = np.zeros((NCORES, ncls, ndch, P, DCHUNK // 16), np.int16)
    d1idx = np.zeros((NCORES, ncls, ndch, P, DCHUNK // 16), np.int16)
    slot_pair = np.full((NCORES, ncls * dsl), -1, np.int64)  # -> original pair
    for c in range(NCORES):
        lo, hi, k = core_cls[c]
        o = np.argsort(k, kind="stable")
        cnt = np.bincount(k, minlength=ncls)
        st = np.concatenate([[0], np.cumsum(cnt)])
        for kk in range(ncls):
            sel = o[st[kk] : st[kk + 1]] + lo
            i0 = np.zeros(dslp, np.int16)
            i1 = np.zeros(dslp, np.int16)
            i0[: len(sel)] = (e0[sel] % segsz).astype(np.int16)
            i1[: len(sel)] = (e1[sel] % segsz).astype(np.int16)
            for ch in range(ndch):
                d0idx[c, kk, ch] = wrap_idx(i0[ch * DCHUNK : (ch + 1) * DCHUNK], DCHUNK)
                d1idx[c, kk, ch] = wrap_idx(i1[ch * DCHUNK : (ch + 1) * DCHUNK], DCHUNK)
            slot_pair[c, kk * dsl : kk * dsl + len(sel)] = sel

    iota_bf = np.broadcast_to(np.arange(P, dtype=np.float32).astype(BF16), (P, P)).copy()
    ident_bf = np.eye(P, dtype=np.float32).astype(BF16)
    b1rep = np.broadcast_to(np.asarray(b1, np.float32), (P, H1)).copy()
    b2rep = np.broadcast_to(
        np.concatenate([np.asarray(b2, np.float32)] * 2), (P, 2 * F2)
    ).copy()
    W1b = np.asarray(W1, np.float32).astype(BF16)
    W2dup = np.concatenate([np.asarray(W2, np.float32)] * 2, axis=1).astype(BF16)

    x = np.asarray(x, np.float32)
    in_maps = []
    for c in range(NCORES):
        xs = np.zeros((csh, F1), np.float32)
        xs[:nsh] = x[c * nsh : (c + 1) * nsh]
        dinv_sh = dinv[c * csh : (c + 1) * csh]
        in_maps.append(
            {
                "xT": xs.T.astype(BF16).copy(),
                "dinvT": dinv_sh.reshape(nblk, P).T.copy(),
                "W1": W1b,
                "W2dup": W2dup,
                "b1rep": b1rep,
                "b2rep": b2rep,
                "iota_bf": iota_bf,
                "ident_bf": ident_bf,
                "sidx": sidx_dev[c * nsup : (c + 1) * nsup],
                "dloc": dloc_dev[c * nsup : (c + 1) * nsup],
                "d0idx": d0idx[c].transpose(0, 2, 1, 3).copy(),
                "d1idx": d1idx[c].transpose(0, 2, 1, 3).copy(),
            }
        )
    meta = dict(
        N=N, F1=F1, H1=H1, F2=F2, nsh=nsh, csh=csh, nblk=nblk, ntot=ntot,
        segsz=segsz, nsup=nsup, tbs=tbs, gsl=gsl, n_call=n_call, nt_sup=nt_sup,
        ncls=ncls, tcls=tcls, dsl=dsl, ndch=ndch, ndec=ndec, ep=ep,
        has_b1=bool(np.any(np.asarray(b1))), has_b2=bool(np.any(np.asarray(b2))),
    )
    return in_maps, meta, slot_pair


# -------------------------------------------------------------- device side


def build(meta, debug=False):
    f32 = mybir.dt.float32
    bf16 = mybir.dt.bfloat16
    i16 = mybir.dt.int16
    csh, nblk, ntot, segsz = meta["csh"], meta["nblk"], meta["ntot"], meta["segsz"]
    F1, H1, F2 = meta["F1"], meta["H1"], meta["F2"]
    nsup, tbs, n_call, nt_sup = meta["nsup"], meta["tbs"], meta["n_call"], meta["nt_sup"]
    ncls, tcls, dsl, ndch = meta["ncls"], meta["tcls"], meta["dsl"], meta["ndch"]
    has_b1, has_b2 = meta["has_b1"], meta["has_b2"]
    F2d = 2 * F2
    AF = mybir.ActivationFunctionType
    nper = SUPB * tbs                     # tiles per (super, seg)

    nc = bacc.Bacc(
        "TRN2", target_bir_lowering=False, debug=debug, num_devices=NCORES,
        num_swdge_queues=4,
    )
    qn = [0]

    def next_q():
        qn[0] = (qn[0] + 1) % 4
        return qn[0]

    xT = nc.dram_tensor("xT", [F1, csh], bf16, kind="ExternalInput")
    dinvT = nc.dram_tensor("dinvT", [P, nblk], f32, kind="ExternalInput")
    W1 = nc.dram_tensor("W1", [F1, H1], bf16, kind="ExternalInput")
    W2dup = nc.dram_tensor("W2dup", [H1, F2d], bf16, kind="ExternalInput")
    b1rep = nc.dram_tensor("b1rep", [P, H1], f32, kind="ExternalInput")
    b2rep = nc.dram_tensor("b2rep", [P, F2d], f32, kind="ExternalInput")
    iota_bf = nc.dram_tensor("iota_bf", [P, P], bf16, kind="ExternalInput")
    ident_bf = nc.dram_tensor("ident_bf", [P, P], bf16, kind="ExternalInput")
    sidx = nc.dram_tensor("sidx", [nsup, P, NSEG, n_call // 16], i16, kind="ExternalInput")
    dloc = nc.dram_tensor("dloc", [nsup, P, nt_sup], bf16, kind="ExternalInput")
    d0idx = nc.dram_tensor("d0idx", [ncls, P, ndch, DCHUNK // 16], i16, kind="ExternalInput")
    d1idx = nc.dram_tensor("d1idx", [ncls, P, ndch, DCHUNK // 16], i16, kind="ExternalInput")
    logits = nc.dram_tensor("logits", [ncls, P, tcls], f32, kind="ExternalOutput")

    h1_shard = nc.dram_tensor("h1_shard", [csh, H1], bf16)
    h1_full = nc.dram_tensor("h1_full", [ntot, H1], bf16, addr_space="Shared")
    h2_shard = nc.dram_tensor("h2_shard", [csh, F2d], bf16)
    h2_full = nc.dram_tensor("h2_full", [ntot, F2d], bf16, addr_space="Shared")
    z_shard = nc.dram_tensor("z_shard", [csh, F2d], bf16)
    z_full = nc.dram_tensor("z_full", [ntot, F2d], bf16, addr_space="Shared")

    rg = [list(range(NCORES))]

    def aggregate(tc, full_tbl, shard_tbl, F, dinv_t, out_tbl, Fout, emit):
        """Per super: one idx DMA + NSEG dma_gathers (per-seg tiles, rr
        queues) + one-hot segment matmuls into per-block PSUM, self-loop
        add, emit(b, t0, epool, ot, bb) filling a per-super staging tile,
        flushed with one DMA to out_tbl."""
        ic = n_call // 16
        with (
            tc.tile_pool(name="idx", bufs=6) as ipool,
            tc.tile_pool(name="gath", bufs=6) as gpool,
            tc.tile_pool(name="oneh", bufs=3) as spool,
            tc.tile_pool(name="dlp", bufs=3) as dpool,
            tc.tile_pool(name="selfl", bufs=4) as hpool,
            tc.tile_pool(name="epi", bufs=4) as epool,
            tc.tile_pool(name="outst", bufs=3) as opool,
            tc.tile_pool(name="ps_acc", bufs=3, space="PSUM") as pacc,
        ):
            for u in range(nsup):
                it = ipool.tile([P, NSEG * ic], i16, tag="it")
                nc.sync.dma_start(
                    out=it[:], in_=sidx[u].rearrange("p s c -> p (s c)")
                )
                gs = []
                for s in range(NSEG):
                    G = gpool.tile([P, nper * F], bf16, tag=f"G{s}")
                    nc.gpsimd.dma_gather(
                        G[:].rearrange("p (t f) -> p t f", t=nper),
                        full_tbl[s * segsz : (s + 1) * segsz, :],
                        it[:, s * ic : (s + 1) * ic],
                        n_call,
                        n_call,
                        F,
                        single_packet=False,
                        queue_num=next_q(),
                    )
                    gs.append(G)
                dt = dpool.tile([P, nt_sup], bf16, tag="dt")
                nc.sync.dma_start(out=dt[:], in_=dloc[u, :, :])
                S = spool.tile([P, nt_sup * P], bf16, tag="S")
                nc.vector.tensor_tensor(
                    out=S[:].rearrange("p (t j) -> p t j", t=nt_sup),
                    in0=dt[:, :, None].to_broadcast([P, nt_sup, P]),
                    in1=iota_t[:, None, :].to_broadcast([P, nt_sup, P]),
                    op=mybir.AluOpType.is_equal,
                )
                hb = hpool.tile([P, SUPB * F], bf16, tag="hblk")
                nc.sync.dma_start(
                    out=hb[:].rearrange("p (b f) -> p b f", b=SUPB),
                    in_=shard_tbl[u * SUPB * P : (u + 1) * SUPB * P, :]
                    .rearrange("(b p) f -> p b f", b=SUPB),
                )
                ot = opool.tile([P, SUPB * Fout], bf16, tag="ot")
                for bb in range(SUPB):
                    b = u * SUPB + bb
                    acc = pacc.tile([P, F], f32, tag=f"acc{bb}")
                    # self-loop row add fused into the PSUM group: I^T @ hb
                    nc.tensor.matmul(
                        out=acc[:],
                        lhsT=ident[:],
                        rhs=hb[:, bb * F : (bb + 1) * F],
                        start=True,
                        stop=False,
                    )
                    ntiles = NSEG * tbs
                    n_i = 0
                    for s in range(NSEG):
                        for j in range(tbs):
                            c = bb * tbs + j
                            t = s * nper + c
                            nc.tensor.matmul(
                                out=acc[:],
                                lhsT=S[:, t * P : (t + 1) * P],
                                rhs=gs[s][:, c * F : (c + 1) * F],
                                start=False,
                                stop=(n_i == ntiles - 1),
                            )
                            n_i += 1
                    emit(u * SUPB + bb, acc, epool, ot, bb)
                nc.sync.dma_start(
                    out=out_tbl[u * SUPB * P : (u + 1) * SUPB * P, :]
                    .rearrange("(b p) f -> p b f", b=SUPB),
                    in_=ot[:].rearrange("p (b f) -> p b f", b=SUPB),
                )

    with tile.TileContext(nc) as tc:
        with tc.tile_pool(name="const", bufs=1) as cpool:
            W1_t = cpool.tile([F1, H1], bf16, tag="w1")
            nc.sync.dma_start(out=W1_t[:], in_=W1[:])
            W2_t = cpool.tile([H1, F2d], bf16, tag="w2")
            nc.sync.dma_start(out=W2_t[:], in_=W2dup[:])
            b1_t = cpool.tile([P, H1], f32, tag="b1")
            nc.sync.dma_start(out=b1_t[:], in_=b1rep[:])
            b2_t = cpool.tile([P, F2d], f32, tag="b2")
            nc.sync.dma_start(out=b2_t[:], in_=b2rep[:])
            dinv_t = cpool.tile([P, nblk], f32, tag="dinv")
            nc.sync.dma_start(out=dinv_t[:], in_=dinvT[:])
            ident = cpool.tile([P, P], bf16, tag="ident")
            nc.sync.dma_start(out=ident[:], in_=ident_bf[:])
            iota_t = cpool.tile([P, P], bf16, tag="iotaf")
            nc.sync.dma_start(out=iota_t[:], in_=iota_bf[:])

            # ---------------- phase A: h1' = (x @ W1) * dinv (sharded GEMM)
            with (
                tc.tile_pool(name="gemm1", bufs=3) as gp,
                tc.tile_pool(name="gemm1x", bufs=1) as gx,
                tc.tile_pool(name="ps_a", bufs=4, space="PSUM") as pa,
            ):
                xT_t = gx.tile([F1, csh], bf16, tag="xT")
                nc.sync.dma_start(out=xT_t[:], in_=xT[:])
                for i in range(nblk):
                    ps = pa.tile([P, H1], f32, tag="psA")
                    nc.tensor.matmul(
                        out=ps[:],
                        lhsT=xT_t[:, i * P : (i + 1) * P],
                        rhs=W1_t[:],
                        start=True,
                        stop=True,
                    )
                    ht = gp.tile([P, H1], bf16, tag="h1t")
                    nc.scalar.activation(
                        out=ht[:], in_=ps[:], func=AF.Copy,
                        scale=dinv_t[:, i : i + 1],
                    )
                    nc.sync.dma_start(
                        out=h1_shard[i * P : (i + 1) * P, :], in_=ht[:]
                    )

            nc.gpsimd.collective_compute(
                "AllGather",
                mybir.AluOpType.bypass,
                ins=[h1_shard.ap().opt()],
                outs=[h1_full.ap().opt()],
                replica_groups=rg,
            )

            # ---------------- phase C: layer-1 aggregation + fused GEMM2
            with (
                tc.tile_pool(name="ps_tr", bufs=1, space="PSUM") as ptr,
                tc.tile_pool(name="ps_h2", bufs=1, space="PSUM") as ph2,
            ):

                def emit1(b, t0, epool, ot, bb):
                    o1 = epool.tile([P, H1], bf16, tag="o1")
                    if has_b1:
                        t1 = epool.tile([P, H1], f32, tag="t1")
                        nc.vector.tensor_scalar_mul(
                            t1[:], t0[:], dinv_t[:, b : b + 1]
                        )
                        t2 = epool.tile([P, H1], f32, tag="t2")
                        nc.vector.tensor_tensor(
                            out=t2[:], in0=t1[:], in1=b1_t[:], op=mybir.AluOpType.add
                        )
                        nc.scalar.activation(out=o1[:], in_=t2[:], func=AF.Relu)
                    else:
                        nc.scalar.activation(
                            out=o1[:], in_=t0[:], func=AF.Relu,
                            scale=dinv_t[:, b : b + 1],
                        )
                    tp = ptr.tile([H1, P], bf16, tag="tp")
                    nc.tensor.transpose(out=tp[:], in_=o1[:], identity=ident[:])
                    o1T = epool.tile([H1, P], bf16, tag="o1T")
                    nc.scalar.activation(out=o1T[:], in_=tp[:], func=AF.Copy)
                    hp = ph2.tile([P, F2d], f32, tag="hp")
                    nc.tensor.matmul(
                        out=hp[:], lhsT=o1T[:], rhs=W2_t[:], start=True, stop=True
                    )
                    nc.scalar.activation(
                        out=ot[:, bb * F2d : (bb + 1) * F2d], in_=hp[:],
                        func=AF.Copy, scale=dinv_t[:, b : b + 1],
                    )

                aggregate(tc, h1_full, h1_shard, H1, dinv_t, h2_shard, F2d, emit1)

            nc.gpsimd.collective_compute(
                "AllGather",
                mybir.AluOpType.bypass,
                ins=[h2_shard.ap().opt()],
                outs=[h2_full.ap().opt()],
                replica_groups=rg,
            )

            # ---------------- phase E: layer-2 aggregation -> z (dup cols)
            def emit2(b, t0, epool, ot, bb):
                if has_b2:
                    t1 = epool.tile([P, F2d], f32, tag="t1z")
                    nc.vector.tensor_scalar_mul(t1[:], t0[:], dinv_t[:, b : b + 1])
                    t2 = epool.tile([P, F2d], f32, tag="t2z")
                    nc.vector.tensor_tensor(
                        out=t2[:], in0=t1[:], in1=b2_t[:], op=mybir.AluOpType.add
                    )
                    nc.vector.tensor_copy(
                        out=ot[:, bb * F2d : (bb + 1) * F2d], in_=t2[:]
                    )
                else:
                    nc.scalar.activation(
                        out=ot[:, bb * F2d : (bb + 1) * F2d], in_=t0[:],
                        func=AF.Copy, scale=dinv_t[:, b : b + 1],
                    )

            aggregate(tc, h2_full, h2_shard, F2d, dinv_t, z_shard, F2d, emit2)

            nc.gpsimd.collective_compute(
                "AllGather",
                mybir.AluOpType.bypass,
                ins=[z_shard.ap().opt()],
                outs=[z_full.ap().opt()],
                replica_groups=rg,
            )

            # ---------------- phase G: decode (16 classes, chunked gathers)
            with (
                tc.tile_pool(name="didx", bufs=8) as ipool,
                tc.tile_pool(name="dz", bufs=2) as zpool,
                tc.tile_pool(name="dm", bufs=2) as mpool,
                tc.tile_pool(name="dl", bufs=3) as lpool,
            ):
                cpt = DCHUNK // P         # tiles per chunk
                dc = DCHUNK // 16
                for k in range(ncls):
                    s0, s1 = k // NSEG, k % NSEG
                    Z0 = zpool.tile([P, tcls * F2d], bf16, tag="Z0")
                    Z1 = zpool.tile([P, tcls * F2d], bf16, tag="Z1")
                    i0 = ipool.tile([P, ndch * dc], i16, tag="i0")
                    nc.sync.dma_start(
                        out=i0[:], in_=d0idx[k].rearrange("p h c -> p (h c)")
                    )
                    i1 = ipool.tile([P, ndch * dc], i16, tag="i1")
                    nc.sync.dma_start(
                        out=i1[:], in_=d1idx[k].rearrange("p h c -> p (h c)")
                    )
                    for ch in range(ndch):
                        nt_ch = min(cpt, tcls - ch * cpt)
                        n_i = nt_ch * P
                        nc.gpsimd.dma_gather(
                            Z0[:, ch * cpt * F2d : (ch * cpt + nt_ch) * F2d]
                            .rearrange("p (t f) -> p t f", t=nt_ch),
                            z_full[s0 * segsz : (s0 + 1) * segsz, :],
                            i0[:, ch * dc : (ch + 1) * dc], n_i, n_i, F2d,
                            single_packet=False, queue_num=next_q(),
                        )
                        nc.gpsimd.dma_gather(
                            Z1[:, ch * cpt * F2d : (ch * cpt + nt_ch) * F2d]
                            .rearrange("p (t f) -> p t f", t=nt_ch),
                            z_full[s1 * segsz : (s1 + 1) * segsz, :],
                            i1[:, ch * dc : (ch + 1) * dc], n_i, n_i, F2d,
                            single_packet=False, queue_num=next_q(),
                        )
                    M = mpool.tile([P, tcls * F2d], bf16, tag="M")
                    nc.vector.tensor_tensor(
                        out=M[:], in0=Z0[:], in1=Z1[:], op=mybir.AluOpType.mult
                    )
                    L = lpool.tile([P, tcls], f32, tag="L")
                    nc.vector.tensor_reduce(
                        out=L[:],
                        in_=M[:].rearrange("p (t f) -> p t f", t=tcls)[:, :, 0:F2],
                        axis=mybir.AxisListType.X,
                        op=mybir.AluOpType.add,
                    )
                    nc.sync.dma_start(out=logits[k, :, :], in_=L[:])

    nc.compile()
    return nc


# -------------------------------------------------------------------- entry


def assemble_logits(results, meta, slot_pair):
    ep = meta["ep"]
    ncls, dsl, tcls = meta["ncls"], meta["dsl"], meta["tcls"]
    logits = np.empty(ep, np.float32)
    for c in range(len(results)):
        lg = results[c]["logits"]  # [ncls, P, tcls]
        vals = lg.transpose(0, 2, 1).reshape(ncls * dsl)  # pos i = j*128+p
        sp = slot_pair[c]
        m = sp >= 0
        logits[sp[m]] = vals[m]
    return logits


def kernel(**inputs) -> np.ndarray:
    in_maps, meta, slot_pair = preprocess(**inputs)
    nc = build(meta)
    res = run_bass_kernel_spmd(nc, in_maps, core_ids=list(range(NCORES)))
    return assemble_logits(res.results, meta, slot_pair)
